# revision 16
# baseline (speedup 1.0000x reference)
"""Trainium2 Bass kernel for patch attention:
    out = softmax(silu(q) @ silu(k)^T * scale, axis=-1)
with q,k: [B=4, H=16, P=1024, D=128] fp32, scale: [1] fp32.

Sharding: B*H = 64 heads split across 8 NeuronCores, 8 heads each.
Output stored as fp16 on device (halves the 32MiB/core store traffic;
softmax values in [0,1], ~0.05% rel err), upcast to fp32 on host.

Measured per-instr costs (on-device microbench) drove the design:
  ACT: activate [128,1024]=1.20us, [128,2048]=2.06us, accum_out +0.28us
  DVE: reduce [128,1024]=1.21us (no 16-bit speedup), stt [128,1024]=1.2us
  GPS: normalize_recip [128,1024] f32->f16 = 1.06us (generic ops useless)
  PE:  f32r transpose 0.33us (vs fp32 0.42), bf16 matmul N=512 0.38us

Per-core, per-head schedule (ACT-bound, ~11.6us/head):
  - q,k loaded as f32r (same bytes as fp32); 16 PE transposes per head
    at the cheaper f32r rate into PSUM; tanh (exp-table-set resident) +
    DVE stt produce bT = 2*silu(x)^T in bf16. The 2x factors fold into
    the exp scale (scale/4).
  - scores for 8 m-tiles as 2 WIDE tiles (m0m1, m4m5 -> one [128,2048]
    exp each, saves ACT instr overhead) + 4 narrow tiles. Row sums: m2
    via ACT accum_out, the rest via DVE tensor_reduce — splits the sum
    work so neither ACT (~92.8us) nor DVE (~87us) exceeds the budget.
  - normalize: all on GpSimd normalize_recip (fp32 in -> fp16 out).
  - PSUM (8 banks): 1x wide score slot [128,2048] (4 banks) + a 2-slot
    [128,1024] pool (4 banks) shared by {xT_k, n2, xT_q, n3, n6, n7} —
    6 allocs/head on 2 slots gives every reuse >=2 ops of drain space,
    and the cyclic tile pattern [W n n W n n] never puts two wide
    tiles back to back on the single wide slot.
  - out DMA per half-head [128,4,1024] fp16 = 1 MiB.
"""

import numpy as np

B, H, P, D = 4, 16, 1024, 128
N_CORES = 8
G = (B * H) // N_CORES  # heads per core = 8
PT = P // 128  # p-tiles per head = 8

USE_F32R_TRANSPOSE = True

_cached = {}


def _build_module():
    import concourse.bass as bass
    import concourse.tile as tile
    from concourse import bacc, mybir

    f32 = mybir.dt.float32
    f32r = mybir.dt.float32r
    f16 = mybir.dt.float16
    bf16 = mybir.dt.bfloat16
    AF = mybir.ActivationFunctionType
    in_dt = f32r if USE_F32R_TRANSPOSE else f32

    nc = bacc.Bacc("TRN2", target_bir_lowering=False, debug=False)
    q_d = nc.dram_tensor("q", [G, P, D], in_dt, kind="ExternalInput")
    k_d = nc.dram_tensor("k", [G, P, D], in_dt, kind="ExternalInput")
    scale_d = nc.dram_tensor("scale", [1], f32, kind="ExternalInput")
    id_d = nc.dram_tensor("ident", [128, 128], in_dt, kind="ExternalInput")
    out_d = nc.dram_tensor("out", [G, P, P], f16, kind="ExternalOutput")

    with tile.TileContext(nc) as tc:
        with (
            tc.tile_pool(name="consts", bufs=1) as consts,
            tc.tile_pool(name="nat", bufs=6) as natp,
            tc.tile_pool(name="th", bufs=3) as thp,
            tc.tile_pool(name="bt", bufs=4) as btp,
            tc.tile_pool(name="expw", bufs=4) as expwp,
            tc.tile_pool(name="expn", bufs=8) as expnp,
            tc.tile_pool(name="outs", bufs=6) as outp,
            tc.tile_pool(name="stats", bufs=24) as statp,
            tc.tile_pool(name="ps_w", bufs=1, space="PSUM") as ps_wp,
            tc.tile_pool(name="ps_nx", bufs=2, space="PSUM") as ps_nxp,
        ):
            identity = consts.tile([128, 128], in_dt)
            nc.sync.dma_start(out=identity, in_=id_d[:, :])
            scale_sb = consts.tile([128, 1], f32)
            nc.gpsimd.dma_start(out=scale_sb, in_=scale_d[:].to_broadcast([128, 1]))
            # bT = 2*silu => scores are 4x; fold the 1/4 into the exp scale.
            # (scale_adj is COMPUTED later, after the first stt ops, so this
            # DVE instruction doesn't sit at the head of the DVE queue
            # waiting on the gpsimd broadcast + lib load during ramp-up.)
            scale_adj = consts.tile([128, 1], f32)

            def dma_in(g, split_queues=False):
                """split_queues: issue q's load on the scalar HWDGE queue so
                it lands in parallel with k's (used for the prologue heads,
                where the serial k-then-q landing is on the critical path;
                steady-state heads prefetch a full head ahead, and a scalar-
                queue dispatch would cost ACT pacer time)."""
                nats = {}
                for nm, src in (("k", k_d), ("q", q_d)):
                    nat = natp.tile(
                        [128, PT, 128], in_dt, tag="nat", name=f"nat_{nm}{g}"
                    )
                    eng = nc.scalar if (split_queues and nm == "q") else nc.sync
                    eng.dma_start(
                        out=nat, in_=src[g].rearrange("(t p) d -> p t d", p=128)
                    )
                    nats[nm] = nat
                return nats

            def transp(g, nat, nm):
                """8 PE transposes of one tensor -> a [128,1024] PSUM slot."""
                ps_x = ps_nxp.tile([128, P], f32, tag="ps_nx", name=f"psX_{nm}{g}")
                dst = ps_x.bitcast(f32r) if USE_F32R_TRANSPOSE else ps_x
                for t in range(PT):
                    nc.tensor.transpose(
                        dst[:, bass.ts(t, 128)], nat[:, t, :], identity
                    )
                return ps_x

            def silu_t(g, ps_x, nm):
                """tanh + stt: bT = (tanh(xT/2)+1)*xT = 2*silu(x)^T, bf16."""
                th = thp.tile([128, P], bf16, tag="th", name=f"th_{nm}{g}")
                nc.scalar.activation(out=th, in_=ps_x, func=AF.Tanh, scale=0.5)
                bt = btp.tile([128, P], bf16, tag=f"bt_{nm}", name=f"bt_{nm}{g}")
                nc.vector.scalar_tensor_tensor(
                    out=bt,
                    in0=th,
                    scalar=1.0,
                    in1=ps_x,
                    op0=mybir.AluOpType.add,
                    op1=mybir.AluOpType.mult,
                )
                return bt

            def mm_tile(ps, col0, btq, btk, m):
                """Two N=512 matmuls filling ps[:, col0:col0+1024] with
                scores of m-tile m."""
                for h in range(2):
                    nc.tensor.matmul(
                        ps[:, col0 + h * 512 : col0 + (h + 1) * 512],
                        btq[:, m * 128 : (m + 1) * 128],
                        btk[:, bass.ts(h, 512)],
                        start=True,
                        stop=True,
                    )

            def norm(g, m, exp_ap, sum_t, out_half):
                nc.gpsimd.normalize_recip(out_half[:, m % 4, :], exp_ap, sum_t)

            def narrow_tile(g, m, btq, btk, out_half, accum, dve_norm, slot=None):
                """One [128,1024] score tile: matmuls + exp (+sums) + norm."""
                slot = m % 4 if slot is None else slot
                ps_n = ps_nxp.tile([128, P], f32, tag="ps_nx", name=f"psN{m}_{g}")
                mm_tile(ps_n, 0, btq, btk, m)
                exp_n = expnp.tile([128, P], f32, tag="expn", name=f"expN{m}_{g}")
                sum_n = statp.tile([128, 1], f32, tag="sum", name=f"sumN{m}_{g}")
                if accum:
                    nc.scalar.activation(
                        out=exp_n, in_=ps_n, func=AF.Exp, scale=scale_adj,
                        accum_out=sum_n,
                    )
                else:
                    nc.scalar.activation(
                        out=exp_n, in_=ps_n, func=AF.Exp, scale=scale_adj
                    )
                    nc.vector.tensor_reduce(
                        out=sum_n, in_=exp_n,
                        axis=mybir.AxisListType.X, op=mybir.AluOpType.add,
                    )
                if dve_norm:
                    rec_n = statp.tile([128, 1], f32, tag="rec", name=f"recN{m}_{g}")
                    nc.vector.reciprocal(rec_n, sum_n)
                    nc.vector.tensor_scalar_mul(
                        out_half[:, slot, :], exp_n, rec_n
                    )
                else:
                    nc.gpsimd.normalize_recip(
                        out_half[:, slot, :], exp_n, sum_n
                    )

            def dma_out(g, half, out_half):
                nc.sync.dma_start(
                    out=out_d[g, half * 512 : (half + 1) * 512, :].rearrange(
                        "(t p) q -> p t q", p=128
                    ),
                    in_=out_half,
                )

            # prologue: head 0 fully prepped, head 1 DMA'd
            nats0 = dma_in(0, split_queues=True)
            bt_k = silu_t(0, transp(0, nats0["k"], "k"), "k")
            bt_q = silu_t(0, transp(0, nats0["q"], "q"), "q")
            nc.vector.tensor_scalar_mul(scale_adj, scale_sb, 0.25)
            nats_next = dma_in(1, split_queues=True)

            for g in range(G):
                if g + 2 < G:
                    nats_after = dma_in(g + 2)
                else:
                    nats_after = None
                out_h0 = outp.tile([128, 4, P], f16, tag="out", name=f"out_{g}_0")
                out_h1 = outp.tile([128, 4, P], f16, tag="out", name=f"out_{g}_1")

                # --- W0: m0, m1 (wide) ---
                ps_w = ps_wp.tile([128, 2 * P], f32, tag="ps_w", name=f"psW0_{g}")
                mm_tile(ps_w, 0, bt_q, bt_k, 0)
                mm_tile(ps_w, P, bt_q, bt_k, 1)
                exp_w = expwp.tile([128, 2 * P], f32, tag="expw", name=f"expW0_{g}")
                nc.scalar.activation(
                    out=exp_w, in_=ps_w, func=AF.Exp, scale=scale_adj
                )
                sums_w = statp.tile([128, 2], f32, tag="sum2", name=f"sumW0_{g}")
                for t in range(2):
                    nc.vector.tensor_reduce(
                        out=sums_w[:, t : t + 1],
                        in_=exp_w[:, bass.ts(t, P)],
                        axis=mybir.AxisListType.X,
                        op=mybir.AluOpType.add,
                    )
                    norm(g, t, exp_w[:, bass.ts(t, P)], sums_w[:, t : t + 1], out_h0)

                # --- prep(g+1): both transpose blocks back to back. With 6
                # ps_nx allocs/head on 2 slots this yields the rotation
                # T_k@A, T_q@B, n2@A, n3@B, n6@A, n7@B, in which every
                # same-slot reuse is separated by >=2 intervening ACT/DVE
                # ops — no forced exp->matmul->exp bubbles on ACT. ---
                if g + 1 < G:
                    ps_xk = transp(g + 1, nats_next["k"], "k")
                    bt_k_next = silu_t(g + 1, ps_xk, "k")
                    ps_xq = transp(g + 1, nats_next["q"], "q")
                    bt_q_next = silu_t(g + 1, ps_xq, "q")

                last = g == G - 1

                # --- n2 (narrow, ACT accum sums) ---
                narrow_tile(g, 2, bt_q, bt_k, out_h0, accum=True, dve_norm=False)

                # --- n3 (narrow; DVE reduce except on the last head) ---
                narrow_tile(g, 3, bt_q, bt_k, out_h0, accum=last, dve_norm=False)
                dma_out(g, 0, out_h0)

                # --- W1: m4, m5 (wide) ---
                ps_w = ps_wp.tile([128, 2 * P], f32, tag="ps_w", name=f"psW1_{g}")
                mm_tile(ps_w, 0, bt_q, bt_k, 4)
                mm_tile(ps_w, P, bt_q, bt_k, 5)
                exp_w = expwp.tile([128, 2 * P], f32, tag="expw", name=f"expW1_{g}")
                nc.scalar.activation(
                    out=exp_w, in_=ps_w, func=AF.Exp, scale=scale_adj
                )
                sums_w = statp.tile([128, 2], f32, tag="sum2", name=f"sumW1_{g}")
                for t in range(2):
                    nc.vector.tensor_reduce(
                        out=sums_w[:, t : t + 1],
                        in_=exp_w[:, bass.ts(t, P)],
                        axis=mybir.AxisListType.X,
                        op=mybir.AluOpType.add,
                    )
                    norm(
                        g, 4 + t, exp_w[:, bass.ts(t, P)], sums_w[:, t : t + 1],
                        out_h1,
                    )

                # --- n6, n7 (narrow, DVE reduce; the last head instead
                # drains via ACT accum + DVE norms + split DMAs into
                # separate tiles to shorten the serial tail) ---
                if not last:
                    narrow_tile(g, 6, bt_q, bt_k, out_h1, accum=False, dve_norm=False)
                    narrow_tile(g, 7, bt_q, bt_k, out_h1, accum=False, dve_norm=False)
                    dma_out(g, 1, out_h1)
                else:
                    nc.sync.dma_start(
                        out=out_d[g, 512:768, :].rearrange(
                            "(t p) q -> p t q", p=128
                        ),
                        in_=out_h1[:, 0:2, :],
                    )
                    for m in (6, 7):
                        oh = outp.tile(
                            [128, 1, P], f16, tag="out1", name=f"out_{g}_m{m}"
                        )
                        narrow_tile(
                            g, m, bt_q, bt_k, oh, accum=True, dve_norm=True,
                            slot=0,
                        )
                        # ACT is already drained here; its HWDGE queue gets
                        # the final small stores so they bypass the sync
                        # queue's in-flight transfers.
                        nc.scalar.dma_start(
                            out=out_d[g, m * 128 : (m + 1) * 128, :],
                            in_=oh[:, 0, :],
                        )

                if g + 1 < G:
                    bt_k, bt_q = bt_k_next, bt_q_next
                    nats_next = nats_after

    nc.compile()
    return nc


def _get_nc():
    if "nc" not in _cached:
        _cached["nc"] = _build_module()
    return _cached["nc"]


def kernel(q, k, scale, _trace=False):
    from concourse.bass_utils import run_bass_kernel_spmd

    nc = _get_nc()
    qf = np.ascontiguousarray(q.reshape(B * H, P, D), dtype=np.float32)
    kf = np.ascontiguousarray(k.reshape(B * H, P, D), dtype=np.float32)
    sc = np.ascontiguousarray(scale.reshape(1), dtype=np.float32)
    ident = np.eye(128, dtype=np.float32)
    in_maps = [
        {
            "q": qf[i * G : (i + 1) * G],
            "k": kf[i * G : (i + 1) * G],
            "scale": sc,
            "ident": ident,
        }
        for i in range(N_CORES)
    ]
    res = run_bass_kernel_spmd(
        nc, in_maps, core_ids=list(range(N_CORES)), trace=_trace
    )
    out = np.concatenate([res.results[i]["out"] for i in range(N_CORES)], axis=0)
    if _trace:
        kernel.last_result = res
    return out.reshape(B, H, P, P).astype(np.float32)
